# revision 33
# baseline (speedup 1.0000x reference)
"""Multi-head attention kernel for Trainium2, SPMD over 8 NeuronCores.

Problem: B=2, S=2048, E=1024, H=16 heads, Dh=64.
  q = per-head q_in @ Wq.T (Wq shared across heads), same for k, v
  attn = softmax(q k^T / 8); ctx = attn @ v; out = concat(ctx) @ Wo.T + bo

Sharding: core c handles batch b=c//4 and heads 4*(c%4)..4*(c%4)+3.
The out projection is sharded by e_out columns (256 rows of Wo per core),
with an AllGather of the per-head normalized attention average over the 4
cores of each batch group in between.

Key structure (bf16 matmuls, fp32 psum; fp8 was tried and rejected — its
per-element quantization noise lands on the output at full strength, ~9%):
  - q/k/wo transposes run on the DMA crossbar (dma_start_transpose), not
    the PE; no psum evacuation copies for them.
  - u = A^T qin^T per head (A = Wq^T Wk) with even/odd heads sharing one
    [128, S] tile at partition bases 0/64, so scores need no hsplit DMAs.
  - attn@v rides a [128, 65] stationary (64 v-dims + ones column -> the
    softmax row-sum accumulates in psum row 64 for free).
  - exp(s/8 - 1) splits between ACT (table exp) and a custom DVE
    squaring-chain op (1 + y/64)^64; the -1 bias centers the weighted
    score distribution so the chain's y^2/128 error mostly cancels.
  - Wv is folded into Wo on chip (WoV = Wo_h @ Wv), removing the ctx
    projection; the AllGather carries normalized attention averages.
  - w2 matmuls trail the score matmuls by 2 chunks so the PE's depth-4
    wait queue never blocks on the exp engines.
"""

import contextlib
import sys

sys.path.insert(0, "/opt/trn_rl_repo")

import numpy as np

import concourse.bass as bass
import concourse.tile as tile
from concourse import bacc, mybir
from concourse.bass_utils import run_bass_kernel_spmd

B, S, E, H, Dh = 2, 2048, 1024, 16, 64
N_CORES = 8
HPC = 4          # heads per core
NK = S // 128    # 16 key chunks
EOUT = E // 4    # e_out columns per core

F32 = mybir.dt.float32
BF16 = mybir.dt.bfloat16

EXP_BIAS = -1.0
EXP_C0 = 0.125 / 64.0
EXP_C1 = 1.0 + EXP_BIAS / 64.0

# of the 16 exp chunks per (head, qh), how many go to the DVE custom op
DVE_EXP_N = 0
W2_LAG = 2

_CACHE = {}
_EXPOP = None


def _register_exp_op():
    """Register the squaring-chain exp as a custom DVE op (append-only)."""
    global _EXPOP
    if _EXPOP is not None:
        return _EXPOP
    import concourse.dve_ops as dvo
    from concourse.dve_spec import Spec, Src0, C0, C1, lower, sq, _has_src1
    from concourse.dve_uop import DveOpSpec

    name = "EXP2X64_ANT"
    if name in dvo._SUB_OPCODE_FOR_NAME:
        _EXPOP = next(op for op in dvo.OPS if op.name == name)
        return _EXPOP

    def _ref(in0, in1, c0, c1, c2):
        t = in0.astype(np.float32) * c0 + c1
        for _ in range(6):
            t = t * t
        return t

    body = Src0 * C0 + C1
    for _ in range(6):
        body = sq(body)
    spec = Spec(body=body, reference=_ref)
    row = dvo._CUSTOM_DVE_ROW_BASE + len(dvo.OPS)
    assert row < 0x20
    shas = {}
    for ver in ("v3", "v4"):
        s = DveOpSpec(
            name=name, opcode=row, uops=lower(spec, ver=ver),
            rd1_en=_has_src1(spec),
        )
        shas[ver] = s.sha(ver)
    op = dvo.DveOp(name, spec, False, shas)
    dvo.OPS.append(op)
    dvo._SUB_OPCODE_FOR_NAME[name] = row
    dvo.CUSTOM_DVE_SPECS[name] = spec
    _EXPOP = op
    return op


def _declare_io(nc):
    io = {}
    io["qin"] = nc.dram_tensor("qin", [S, HPC * Dh], F32, kind="ExternalInput").ap()
    io["kin"] = nc.dram_tensor("kin", [S, HPC * Dh], F32, kind="ExternalInput").ap()
    io["vin"] = nc.dram_tensor("vin", [S, HPC * Dh], F32, kind="ExternalInput").ap()
    io["wq"] = nc.dram_tensor("wq", [Dh, Dh], F32, kind="ExternalInput").ap()
    io["wk"] = nc.dram_tensor("wk", [Dh, Dh], F32, kind="ExternalInput").ap()
    io["wv"] = nc.dram_tensor("wv", [Dh, Dh], F32, kind="ExternalInput").ap()
    io["wo_s"] = nc.dram_tensor("wo_s", [EOUT, E], F32, kind="ExternalInput").ap()
    io["bo_s"] = nc.dram_tensor("bo_s", [2, 128], F32, kind="ExternalInput").ap()
    io["outT"] = nc.dram_tensor("outT", [EOUT, S], F32, kind="ExternalOutput").ap()
    return io


def _body(nc, tc, es, io, it, collective=True):
    expop = _register_exp_op()

    def pool(name, bufs, space="SBUF"):
        return es.enter_context(
            tc.tile_pool(name=f"{name}_{it}", bufs=bufs, space=space)
        )

    qin, kin, vin = io["qin"], io["kin"], io["vin"]
    wq, wk, wv, wo_s, bo_s, outT = (
        io["wq"], io["wk"], io["wv"], io["wo_s"], io["bo_s"], io["outT"],
    )

    persist = pool("persist", 1)
    stage = pool("stage", 3)
    ppool = pool("pp", 4)
    npool = pool("np", 2)
    opool = pool("op", 2)
    psB = pool("psB", 2, space="PSUM")     # [128, 1024] f32 slots (2 banks x2)
    psW = pool("psW", 2, space="PSUM")     # [65, 1024] f32 slots (2 banks x2)
    dram = pool("dram", 1, space="DRAM")

    # ---------------- tiny weights ----------------
    wq_sb = persist.tile([Dh, Dh], F32, tag="wq_sb")
    nc.sync.dma_start(out=wq_sb[:], in_=wq[:, :])
    wk_sb = persist.tile([Dh, Dh], F32, tag="wk_sb")
    nc.sync.dma_start(out=wk_sb[:], in_=wk[:, :])
    wv_sb = persist.tile([Dh, Dh], F32, tag="wv_sb")
    nc.sync.dma_start(out=wv_sb[:], in_=wv[:, :])
    wq_bf = persist.tile([Dh, Dh], BF16, tag="wq_bf")
    nc.vector.tensor_copy(wq_bf[:], wq_sb[:])
    wk_bf = persist.tile([Dh, Dh], BF16, tag="wk_bf")
    nc.vector.tensor_copy(wk_bf[:], wk_sb[:])

    # A = Wq^T @ Wk duplicated on both partition halves
    a_ps = psB.tile([Dh, Dh], F32, tag="sc", name=f"aps_{it}")
    nc.tensor.matmul(a_ps[:], wq_bf[:], wk_bf[:], start=True, stop=True)
    a2 = persist.tile([128, Dh], BF16, tag="a2")
    nc.vector.tensor_copy(a2[0:Dh, :], a_ps[:])
    nc.sync.dma_start(out=a2[Dh:128, :], in_=a2[0:Dh, :])

    # wv duplicated on both partition halves (for the WoV fold)
    wv2 = persist.tile([128, Dh], BF16, tag="wv2")
    nc.vector.tensor_copy(wv2[0:Dh, :], wv_sb[:])
    nc.sync.dma_start(out=wv2[Dh:128, :], in_=wv2[0:Dh, :])

    bo_sb = persist.tile([128, 2], F32, tag="bo_sb")
    for hh in range(2):
        nc.sync.dma_start(
            out=bo_sb[:, hh : hh + 1],
            in_=bo_s[hh, :].rearrange("(p one) -> p one", one=1),
        )
    nb1 = persist.tile([128, 1], F32, tag="nb1")
    nc.vector.memset(nb1[:], EXP_BIAS)

    # ---------------- q/k: load f32, cast bf16 per pack, xbar transpose ----
    # pack g holds heads 2g (partitions 0..64) and 2g+1 (64..128)
    qT = [persist.tile([128, NK, 128], BF16, tag=f"qT{g}", name=f"qT{g}") for g in range(2)]
    kT = [persist.tile([128, NK, 128], BF16, tag=f"kT{g}", name=f"kT{g}") for g in range(2)]
    stq = [persist.tile([128, NK, 128], BF16, tag=f"stq{g}", name=f"stq{g}") for g in range(2)]
    stk = [persist.tile([128, NK, 128], BF16, tag=f"stk{g}", name=f"stk{g}") for g in range(2)]

    # u2[g]: [128, S] bf16, head 2g rows 0..64, head 2g+1 rows 64..128
    u2 = [persist.tile([128, S], BF16, tag=f"u2{g}", name=f"u2{g}") for g in range(2)]
    vin_ones = persist.tile([128, NK, HPC, Dh + 1], BF16, tag="vin_ones")
    for j in range(HPC):
        nc.vector.memset(vin_ones[:, :, j, Dh : Dh + 1], 1.0)

    # All big loads issue back-to-back on the SP queue, ordered by first use:
    # q half0 (longest dependent chain), k half0, v half0, k half1, v half1,
    # q half1. Transposes go out on the ACT hwdge queue so they never block
    # a load behind their cast dependency.
    ldq, ldk, ldv = {}, {}, {}

    def emit_load(dst_map, src, h, label):
        st = stage.tile([128, NK // 2, 256], F32, tag="qkstage",
                        name=f"st_{it}_{label}_{h}")
        nc.sync.dma_start(
            out=st[:],
            in_=src[1024 * h : 1024 * (h + 1), :].rearrange("(c p) d -> p c d", p=128),
        )
        dst_map[h] = st

    def emit_qk_half(st, stp, dstT, h, engs):
        for g in range(2):
            engs[g](
                stp[g][:, 8 * h : 8 * (h + 1), :],
                st[:, :, 128 * g : 128 * (g + 1)],
            )
            nc.scalar.dma_start_transpose(
                dstT[g][:, 8 * h : 8 * (h + 1), :],
                stp[g][:, 8 * h : 8 * (h + 1), :],
            )

    def emit_u_half(h):
        for g in range(2):
            for t in range(2 * h, 2 * h + 2):
                u_ps = psB.tile([128, 512], F32, tag="sc", name=f"ups_{it}_{g}_{t}")
                for par in range(2):
                    sl = slice(64 * par, 64 * (par + 1))
                    nc.tensor.matmul(
                        u_ps[sl, :],
                        a2[sl, :],
                        qT[g][sl, 4 * t : 4 * (t + 1), :],
                        start=True, stop=True,
                    )
                nc.vector.tensor_copy(u2[g][:, 512 * t : 512 * (t + 1)], u_ps[:])

    def emit_v_half(h, stv):
        cast_eng = [nc.vector.tensor_copy, nc.gpsimd.tensor_copy]
        for j in range(HPC):
            cast_eng[(2 * h + j) % 2](
                vin_ones[:, 8 * h : 8 * (h + 1), j, 0:Dh],
                stv[:, :, 64 * j : 64 * (j + 1)],
            )

    emit_load(ldq, qin, 0, "q")
    emit_load(ldk, kin, 0, "k")
    emit_qk_half(ldq[0], stq, qT, 0, [nc.vector.tensor_copy, nc.gpsimd.tensor_copy])
    emit_qk_half(ldk[0], stk, kT, 0, [nc.gpsimd.tensor_copy, nc.vector.tensor_copy])
    emit_u_half(0)
    emit_load(ldv, vin, 0, "v")
    emit_load(ldk, kin, 1, "k")
    emit_v_half(0, ldv[0])
    emit_qk_half(ldk[1], stk, kT, 1, [nc.gpsimd.tensor_copy, nc.vector.tensor_copy])
    emit_load(ldv, vin, 1, "v")
    emit_load(ldq, qin, 1, "q")
    emit_v_half(1, ldv[1])
    emit_qk_half(ldq[1], stq, qT, 1, [nc.vector.tensor_copy, nc.gpsimd.tensor_copy])
    emit_u_half(1)

    # ---------------- wo: cast-DMA + xbar transpose + Wv fold --------------
    wo_bf = persist.tile([128, 2, E], BF16, tag="wo_bf")
    nc.gpsimd.dma_start(
        out=wo_bf[:], in_=wo_s.rearrange("(rc p) e -> p rc e", p=128)
    )
    woT_t = persist.tile([128, 16, 128], BF16, tag="woT_t")
    nc.sync.dma_start_transpose(woT_t[:], wo_bf[:])
    # woT[c8][x, 128*rc + t] = woT_t[x, rc*8 + c8, t]
    woVT = persist.tile([128, 8, EOUT], BF16, tag="woVT")
    for c8 in range(8):
        wv_ps = psB.tile([128, 2, 128], F32, tag="sc", name=f"wvps_{it}_{c8}")
        for hh in range(2):
            sl = slice(64 * hh, 64 * (hh + 1))
            nc.tensor.matmul(
                wv_ps[sl, :, :],
                wv2[sl, :],
                woT_t[sl, :, :].rearrange("p (rc c) t -> p c rc t", rc=2)[:, c8, :, :],
                start=True, stop=True,
            )
        nc.scalar.copy(woVT[:, c8, :], wv_ps[:])

    # ---------------- AG staging ----------------
    in_cc = dram.tile([2 * Dh, S], BF16)  # heads 0,1
    in_cc2h = {
        (j, qh): dram.tile([Dh, S // 2], BF16, name=f"incc2_{it}_{j}_{qh}",
                           tag=f"incc2{j}{qh}")
        for j in (2, 3) for qh in range(2)
    }
    ag_outs = [
        dram.tile([512, S], BF16, addr_space="Local",
                  name=f"agout_{it}_{w}", tag=f"agout{w}")
        for w in range(2)
    ]
    ag2h = {
        (j, qh): dram.tile([4 * Dh, S // 2], BF16, addr_space="Local",
                           name=f"ag2h_{it}_{j}_{qh}", tag=f"ag2h{j}{qh}")
        for j in (2, 3) for qh in range(2)
    }
    agch = pool("agch", 1)
    cch = {
        c8: agch.tile([128, S], BF16, tag=f"ag{c8}", name=f"ag{c8}_{it}")
        for c8 in range(0, 8, 2)
    }
    cch_odd = [
        [agch.tile([128, S // 2], BF16, tag=f"agodd{r}_{h}", name=f"agodd{r}_{h}_{it}")
         for h in range(2)]
        for r in range(4)
    ]
    o_acc = [opool.tile([128, S], F32, tag=f"oacc{h}", bufs=1, name=f"oacc{h}_{it}")
             for h in range(2)]

    def emit_ag(which):
        if collective:
            nc.gpsimd.collective_compute(
                "AllGather",
                mybir.AluOpType.bypass,
                replica_groups=[[0, 1, 2, 3], [4, 5, 6, 7]],
                ins=[in_cc[:, :].opt()],
                outs=[ag_outs[which].opt()],
            )
        else:
            nc.sync.dma_start(out=ag_outs[which][0:128, :], in_=in_cc[:, :])
        for r in range(4):
            c8 = 2 * r + which
            nc.sync.dma_start(out=cch[c8][:], in_=ag_outs[which][128 * r : 128 * (r + 1), :])

    def emit_ag2(j, qh):
        """AllGather one head's q-half right after its normalize."""
        key = (j, qh)
        if collective:
            nc.gpsimd.collective_compute(
                "AllGather",
                mybir.AluOpType.bypass,
                replica_groups=[[0, 1, 2, 3], [4, 5, 6, 7]],
                ins=[in_cc2h[key][:, :].opt()],
                outs=[ag2h[key].opt()],
            )
        else:
            nc.sync.dma_start(out=ag2h[key][0:Dh, :], in_=in_cc2h[key][:, :])
        for r in range(4):
            nc.sync.dma_start(
                out=cch_odd[r][qh][Dh * (j - 2) : Dh * (j - 1), :],
                in_=ag2h[key][Dh * r : Dh * (r + 1), :],
            )

    # ---------------- attention ----------------
    w2_tiles = {}

    def emit_head_qh(j, qh, hooks):
        """scores -> exp -> (lagged) w2 accumulation for one (head, q-half)."""
        g, par = j // 2, j % 2
        sl = slice(64 * par, 64 * (par + 1))
        w2_ps = psW.tile([Dh + 1, S // 2], F32, tag="w2", name=f"w2ps_{it}_{j}_{qh}")
        w2_tiles[(j, qh)] = w2_ps
        p_tiles = {}

        def emit_w2(m):
            p_bf = p_tiles.pop(m)
            for u in range(2):
                nc.tensor.matmul(
                    w2_ps[:, 512 * u : 512 * (u + 1)],
                    vin_ones[:, m, j, :],
                    p_bf[:, 512 * u : 512 * (u + 1)],
                    start=(m == 0), stop=(m == NK - 1),
                )

        for m in range(NK):
            if m in hooks:
                hooks[m]()
            sc = psB.tile([128, S // 2], F32, tag="sc", name=f"sc_{it}_{j}_{qh}_{m}")
            for u in range(2):
                nc.tensor.matmul(
                    sc[:, 512 * u : 512 * (u + 1)],
                    kT[g][sl, m, :],
                    u2[g][sl, 1024 * qh + 512 * u : 1024 * qh + 512 * (u + 1)],
                    start=True, stop=True,
                )
            p_bf = ppool.tile([128, S // 2], BF16, tag="p", name=f"p_{it}_{j}_{qh}_{m}")
            p_tiles[m] = p_bf
            use_dve = ((m + 1) * DVE_EXP_N) // NK != (m * DVE_EXP_N) // NK
            if use_dve:
                nc.vector._custom_dve(
                    expop, out=p_bf[:], in0=sc[:], s0=EXP_C0, s1=EXP_C1,
                )
            else:
                nc.scalar.activation(
                    p_bf[:], sc[:], mybir.ActivationFunctionType.Exp,
                    scale=0.125, bias=nb1[:],
                )
            if m >= W2_LAG:
                emit_w2(m - W2_LAG)
        for m in range(NK - W2_LAG, NK):
            emit_w2(m)

    def emit_norm(j, qh):
        """reciprocal of the row-sum, broadcast, normalize, ship to DRAM."""
        w2_ps = w2_tiles.pop((j, qh))
        rs_sb = npool.tile([1, S // 2], F32, tag="rs_sb", name=f"rss_{it}_{j}_{qh}")
        nc.vector.tensor_copy(rs_sb[:], w2_ps[Dh : Dh + 1, :])
        rsr = npool.tile([1, S // 2], F32, tag="rsr", name=f"rsr_{it}_{j}_{qh}")
        nc.vector.reciprocal_approx_fast(out=rsr[:], in_=rs_sb[:])
        rs_b = npool.tile([Dh, S // 2], F32, tag="rs_b", name=f"rsb_{it}_{j}_{qh}")
        nc.gpsimd.partition_broadcast(rs_b[:], rsr[:])
        w2n = npool.tile([Dh, S // 2], BF16, tag="w2n", name=f"w2n_{it}_{j}_{qh}")
        nc.vector.tensor_mul(w2n[:], w2_ps[0:Dh, :], rs_b[:])
        if j < 2:
            nc.sync.dma_start(
                out=in_cc[Dh * j : Dh * (j + 1), 1024 * qh : 1024 * (qh + 1)],
                in_=w2n[:],
            )
        else:
            nc.sync.dma_start(
                out=in_cc2h[(j, qh)][:, :], in_=w2n[:]
            )

    def emit_oproj_group(round_, sh, h):
        o_ps = psB.tile([128, 1024], F32, tag="sc", name=f"ops_{it}_{round_}_{h}_{sh}")
        for i, r in enumerate(range(4)):
            c8 = 2 * r + round_
            for u in range(2):
                rhs = (
                    cch[c8][:, 1024 * sh + 512 * u : 1024 * sh + 512 * (u + 1)]
                    if round_ == 0
                    else cch_odd[r][sh][:, 512 * u : 512 * (u + 1)]
                )
                nc.tensor.matmul(
                    o_ps[:, 512 * u : 512 * (u + 1)],
                    woVT[:, c8, 128 * h : 128 * (h + 1)],
                    rhs,
                    start=(i == 0), stop=(i == 3),
                )
        if round_ == 0:
            nc.vector.tensor_copy(o_acc[h][:, 1024 * sh : 1024 * (sh + 1)], o_ps[:])
        else:
            o_sb = opool.tile([128, 1024], F32, tag="osb", name=f"osb_{it}_{h}_{sh}")
            nc.vector.scalar_tensor_tensor(
                o_sb[:], o_ps[:], bo_sb[:, h : h + 1],
                o_acc[h][:, 1024 * sh : 1024 * (sh + 1)],
                mybir.AluOpType.add, mybir.AluOpType.add,
            )
            nc.sync.dma_start(
                out=outT[128 * h : 128 * (h + 1), 1024 * sh : 1024 * (sh + 1)],
                in_=o_sb[:],
            )

    def emit_oproj(round_):
        for sh in range(2):
            for h in range(2):
                emit_oproj_group(round_, sh, h)

    # software pipeline: normalize of (j, qh) is emitted mid-way through the
    # following (head, q-half); AG0 follows head 1; oproj round 0 overlaps
    # head 3's first q-half.
    seq = [(j, qh) for j in range(HPC) for qh in range(2)]
    for idx, (j, qh) in enumerate(seq):
        hooks = {}
        if idx > 0:
            prev = seq[idx - 1]
            def mk(prev=prev):
                def f():
                    emit_norm(*prev)
                    if prev == (1, 1):
                        emit_ag(0)
                    if prev[0] >= 2:
                        emit_ag2(*prev)
                return f
            hooks[6] = mk()
        if (j, qh) == (3, 0):
            hooks[10] = lambda: emit_oproj(0)
        emit_head_qh(j, qh, hooks)
    emit_norm(3, 1)
    emit_ag2(3, 1)
    for h in range(2):
        emit_oproj_group(1, 0, h)
    for h in range(2):
        emit_oproj_group(1, 1, h)


def _build(repeats=1, collective=True):
    key = (repeats, collective)
    if key in _CACHE:
        return _CACHE[key]
    _register_exp_op()
    ndev = N_CORES if collective else 1
    nc = bacc.Bacc("TRN2", target_bir_lowering=False, debug=False, num_devices=ndev)
    io = _declare_io(nc)
    with tile.TileContext(nc) as tc:
        for it in range(repeats):
            with contextlib.ExitStack() as es:
                _body(nc, tc, es, io, it, collective=collective)
    nc.compile()
    _CACHE[key] = nc
    return nc


def kernel(k_in, q_in, v_in, Wq, Wk, Wv, Wo, bo, _repeats=1, _results_hook=None):
    k_in = np.asarray(k_in, dtype=np.float32)
    q_in = np.asarray(q_in, dtype=np.float32)
    v_in = np.asarray(v_in, dtype=np.float32)
    Wq = np.ascontiguousarray(np.asarray(Wq, dtype=np.float32))
    Wk = np.ascontiguousarray(np.asarray(Wk, dtype=np.float32))
    Wv = np.ascontiguousarray(np.asarray(Wv, dtype=np.float32))
    Wo = np.asarray(Wo, dtype=np.float32)
    bo = np.asarray(bo, dtype=np.float32)

    nc = _build(_repeats)

    in_maps = []
    for c in range(N_CORES):
        b, q4 = c // 4, c % 4
        sl = slice(256 * q4, 256 * (q4 + 1))
        in_maps.append(
            {
                "qin": np.ascontiguousarray(q_in[b, :, sl]),
                "kin": np.ascontiguousarray(k_in[b, :, sl]),
                "vin": np.ascontiguousarray(v_in[b, :, sl]),
                "wq": Wq,
                "wk": Wk,
                "wv": Wv,
                "wo_s": np.ascontiguousarray(Wo[sl, :]),
                "bo_s": np.ascontiguousarray(bo[sl].reshape(2, 128)),
            }
        )

    res = run_bass_kernel_spmd(nc, in_maps, core_ids=list(range(N_CORES)))
    if _results_hook is not None:
        _results_hook(res)

    out = np.empty((B, S, E), dtype=np.float32)
    for c in range(N_CORES):
        b, q4 = c // 4, c % 4
        out[b, :, 256 * q4 : 256 * (q4 + 1)] = res.results[c]["outT"].T
    return out


# revision 36
# speedup vs baseline: 1.0666x; 1.0666x over previous
"""Multi-head attention kernel for Trainium2, SPMD over 8 NeuronCores.

Problem: B=2, S=2048, E=1024, H=16 heads, Dh=64.
  q = per-head q_in @ Wq.T (Wq shared across heads), same for k, v
  attn = softmax(q k^T / 8); ctx = attn @ v; out = concat(ctx) @ Wo.T + bo

Sharding: core c handles batch b=c//4 and heads 4*(c%4)..4*(c%4)+3.
The out projection is sharded by e_out columns (256 rows of Wo per core),
with an AllGather of the per-head normalized attention average over the 4
cores of each batch group in between.

Key structure (bf16 matmuls, fp32 psum; fp8 was tried and rejected — its
per-element quantization noise lands on the output at full strength, ~9%):
  - q/k/wo transposes run on the DMA crossbar (dma_start_transpose), not
    the PE; no psum evacuation copies for them.
  - u = A^T qin^T per head (A = Wq^T Wk) with even/odd heads sharing one
    [128, S] tile at partition bases 0/64, so scores need no hsplit DMAs.
  - attn@v rides a [128, 65] stationary (64 v-dims + ones column -> the
    softmax row-sum accumulates in psum row 64 for free).
  - exp(s/8 - 1) splits between ACT (table exp) and a custom DVE
    squaring-chain op (1 + y/64)^64; the -1 bias centers the weighted
    score distribution so the chain's y^2/128 error mostly cancels.
  - Wv is folded into Wo on chip (WoV = Wo_h @ Wv), removing the ctx
    projection; the AllGather carries normalized attention averages.
  - w2 matmuls trail the score matmuls by 2 chunks so the PE's depth-4
    wait queue never blocks on the exp engines.
"""

import contextlib
import sys

sys.path.insert(0, "/opt/trn_rl_repo")

import numpy as np

import concourse.bass as bass
import concourse.tile as tile
from concourse import bacc, mybir
from concourse.bass_utils import run_bass_kernel_spmd

B, S, E, H, Dh = 2, 2048, 1024, 16, 64
N_CORES = 8
HPC = 4          # heads per core
NK = S // 128    # 16 key chunks
EOUT = E // 4    # e_out columns per core

F32 = mybir.dt.float32
BF16 = mybir.dt.bfloat16

EXP_BIAS = -1.0
EXP_C0 = 0.125 / 64.0
EXP_C1 = 1.0 + EXP_BIAS / 64.0

# of the 16 exp chunks per (head, qh), how many go to the DVE custom op
DVE_EXP_N = 0
W2_LAG = 2

_CACHE = {}
_EXPOP = None


def _register_exp_op():
    """Register the squaring-chain exp as a custom DVE op (append-only)."""
    global _EXPOP
    if _EXPOP is not None:
        return _EXPOP
    import concourse.dve_ops as dvo
    from concourse.dve_spec import Spec, Src0, C0, C1, lower, sq, _has_src1
    from concourse.dve_uop import DveOpSpec

    name = "EXP2X64_ANT"
    if name in dvo._SUB_OPCODE_FOR_NAME:
        _EXPOP = next(op for op in dvo.OPS if op.name == name)
        return _EXPOP

    def _ref(in0, in1, c0, c1, c2):
        t = in0.astype(np.float32) * c0 + c1
        for _ in range(6):
            t = t * t
        return t

    body = Src0 * C0 + C1
    for _ in range(6):
        body = sq(body)
    spec = Spec(body=body, reference=_ref)
    row = dvo._CUSTOM_DVE_ROW_BASE + len(dvo.OPS)
    assert row < 0x20
    shas = {}
    for ver in ("v3", "v4"):
        s = DveOpSpec(
            name=name, opcode=row, uops=lower(spec, ver=ver),
            rd1_en=_has_src1(spec),
        )
        shas[ver] = s.sha(ver)
    op = dvo.DveOp(name, spec, False, shas)
    dvo.OPS.append(op)
    dvo._SUB_OPCODE_FOR_NAME[name] = row
    dvo.CUSTOM_DVE_SPECS[name] = spec
    _EXPOP = op
    return op


def _declare_io(nc):
    io = {}
    io["qin"] = nc.dram_tensor("qin", [S, HPC * Dh], F32, kind="ExternalInput").ap()
    io["kin"] = nc.dram_tensor("kin", [S, HPC * Dh], F32, kind="ExternalInput").ap()
    io["vin"] = nc.dram_tensor("vin", [S, HPC * Dh], F32, kind="ExternalInput").ap()
    io["wq"] = nc.dram_tensor("wq", [Dh, Dh], F32, kind="ExternalInput").ap()
    io["wk"] = nc.dram_tensor("wk", [Dh, Dh], F32, kind="ExternalInput").ap()
    io["wv"] = nc.dram_tensor("wv", [Dh, Dh], F32, kind="ExternalInput").ap()
    io["wo_s"] = nc.dram_tensor("wo_s", [EOUT, E], F32, kind="ExternalInput").ap()
    io["bo_s"] = nc.dram_tensor("bo_s", [2, 128], F32, kind="ExternalInput").ap()
    io["outT"] = nc.dram_tensor("outT", [EOUT, S], F32, kind="ExternalOutput").ap()
    return io


def _body(nc, tc, es, io, it, collective=True):
    expop = _register_exp_op()

    def pool(name, bufs, space="SBUF"):
        return es.enter_context(
            tc.tile_pool(name=f"{name}_{it}", bufs=bufs, space=space)
        )

    qin, kin, vin = io["qin"], io["kin"], io["vin"]
    wq, wk, wv, wo_s, bo_s, outT = (
        io["wq"], io["wk"], io["wv"], io["wo_s"], io["bo_s"], io["outT"],
    )

    persist = pool("persist", 1)
    stage = pool("stage", 3)
    ppool = pool("pp", 4)
    npool = pool("np", 2)
    opool = pool("op", 2)
    psB = pool("psB", 2, space="PSUM")     # [128, 1024] f32 slots (2 banks x2)
    psW = pool("psW", 2, space="PSUM")     # [65, 1024] f32 slots (2 banks x2)
    dram = pool("dram", 1, space="DRAM")

    # ---------------- tiny weights ----------------
    wq_sb = persist.tile([Dh, Dh], F32, tag="wq_sb")
    nc.sync.dma_start(out=wq_sb[:], in_=wq[:, :])
    wk_sb = persist.tile([Dh, Dh], F32, tag="wk_sb")
    nc.sync.dma_start(out=wk_sb[:], in_=wk[:, :])
    wv_sb = persist.tile([Dh, Dh], F32, tag="wv_sb")
    nc.sync.dma_start(out=wv_sb[:], in_=wv[:, :])
    wq_bf = persist.tile([Dh, Dh], BF16, tag="wq_bf")
    nc.vector.tensor_copy(wq_bf[:], wq_sb[:])
    wk_bf = persist.tile([Dh, Dh], BF16, tag="wk_bf")
    nc.vector.tensor_copy(wk_bf[:], wk_sb[:])

    # A = Wq^T @ Wk duplicated on both partition halves
    a_ps = psB.tile([Dh, Dh], F32, tag="sc", name=f"aps_{it}")
    nc.tensor.matmul(a_ps[:], wq_bf[:], wk_bf[:], start=True, stop=True)
    a2 = persist.tile([128, Dh], BF16, tag="a2")
    nc.vector.tensor_copy(a2[0:Dh, :], a_ps[:])
    nc.scalar.dma_start(out=a2[Dh:128, :], in_=a2[0:Dh, :])

    # wv duplicated on both partition halves (for the WoV fold)
    wv2 = persist.tile([128, Dh], BF16, tag="wv2")
    nc.vector.tensor_copy(wv2[0:Dh, :], wv_sb[:])
    nc.scalar.dma_start(out=wv2[Dh:128, :], in_=wv2[0:Dh, :])

    bo_sb = persist.tile([128, 2], F32, tag="bo_sb")
    for hh in range(2):
        nc.scalar.dma_start(
            out=bo_sb[:, hh : hh + 1],
            in_=bo_s[hh, :].rearrange("(p one) -> p one", one=1),
        )
    nb1 = persist.tile([128, 1], F32, tag="nb1")
    nc.vector.memset(nb1[:], EXP_BIAS)

    # ---------------- q/k: load f32, cast bf16 per pack, xbar transpose ----
    # pack g holds heads 2g (partitions 0..64) and 2g+1 (64..128)
    qT = [persist.tile([128, NK, 128], BF16, tag=f"qT{g}", name=f"qT{g}") for g in range(2)]
    kT = [persist.tile([128, NK, 128], BF16, tag=f"kT{g}", name=f"kT{g}") for g in range(2)]
    stq = [persist.tile([128, NK, 128], BF16, tag=f"stq{g}", name=f"stq{g}") for g in range(2)]
    stk = [persist.tile([128, NK, 128], BF16, tag=f"stk{g}", name=f"stk{g}") for g in range(2)]

    # u2[g]: [128, S] bf16, head 2g rows 0..64, head 2g+1 rows 64..128
    u2 = [persist.tile([128, S], BF16, tag=f"u2{g}", name=f"u2{g}") for g in range(2)]
    vin_ones = persist.tile([128, NK, HPC, Dh + 1], BF16, tag="vin_ones")
    for j in range(HPC):
        nc.vector.memset(vin_ones[:, :, j, Dh : Dh + 1], 1.0)

    # All big loads issue back-to-back on the SP queue, ordered by first use:
    # q half0 (longest dependent chain), k half0, v half0, k half1, v half1,
    # q half1. Transposes go out on the ACT hwdge queue so they never block
    # a load behind their cast dependency.
    ldq, ldk, ldv = {}, {}, {}

    def emit_load(dst_map, src, h, label):
        st = stage.tile([128, NK // 2, 256], F32, tag="qkstage",
                        name=f"st_{it}_{label}_{h}")
        nc.sync.dma_start(
            out=st[:],
            in_=src[1024 * h : 1024 * (h + 1), :].rearrange("(c p) d -> p c d", p=128),
        )
        dst_map[h] = st

    def emit_qk_half(st, stp, dstT, h, engs):
        for g in range(2):
            engs[g](
                stp[g][:, 8 * h : 8 * (h + 1), :],
                st[:, :, 128 * g : 128 * (g + 1)],
            )
            nc.scalar.dma_start_transpose(
                dstT[g][:, 8 * h : 8 * (h + 1), :],
                stp[g][:, 8 * h : 8 * (h + 1), :],
            )

    def emit_u_half(h):
        for g in range(2):
            for t in range(2 * h, 2 * h + 2):
                u_ps = psB.tile([128, 512], F32, tag="sc", name=f"ups_{it}_{g}_{t}")
                for par in range(2):
                    sl = slice(64 * par, 64 * (par + 1))
                    nc.tensor.matmul(
                        u_ps[sl, :],
                        a2[sl, :],
                        qT[g][sl, 4 * t : 4 * (t + 1), :],
                        start=True, stop=True,
                    )
                nc.vector.tensor_copy(u2[g][:, 512 * t : 512 * (t + 1)], u_ps[:])

    def emit_v_half(h, stv):
        cast_eng = [nc.vector.tensor_copy, nc.gpsimd.tensor_copy]
        for j in range(HPC):
            cast_eng[(2 * h + j) % 2](
                vin_ones[:, 8 * h : 8 * (h + 1), j, 0:Dh],
                stv[:, :, 64 * j : 64 * (j + 1)],
            )

    def emit_load_q(dst_map, src, h, quarter, label):
        """Quarter-load ([128, 4, 256], ~2.9us) for fast prologue starts."""
        st = dst_map.get(h)
        if st is None:
            st = stage.tile([128, NK // 2, 256], F32, tag="qkstage",
                            name=f"st_{it}_{label}_{h}")
            dst_map[h] = st
        o = 1024 * h + 512 * quarter
        nc.sync.dma_start(
            out=st[:, 4 * quarter : 4 * (quarter + 1), :],
            in_=src[o : o + 512, :].rearrange("(c p) d -> p c d", p=128),
        )

    def emit_qk_quarter(st, stp, dstT, h, quarter, engs):
        cs = slice(8 * h + 4 * quarter, 8 * h + 4 * (quarter + 1))
        for g in range(2):
            engs[g](
                stp[g][:, cs, :],
                st[:, 4 * quarter : 4 * (quarter + 1), 128 * g : 128 * (g + 1)],
            )
            nc.scalar.dma_start_transpose(dstT[g][:, cs, :], stp[g][:, cs, :])

    def emit_u_t(g, t):
        u_ps = psB.tile([128, 512], F32, tag="sc", name=f"ups_{it}_{g}_{t}")
        for par in range(2):
            sl = slice(64 * par, 64 * (par + 1))
            nc.tensor.matmul(
                u_ps[sl, :],
                a2[sl, :],
                qT[g][sl, 4 * t : 4 * (t + 1), :],
                start=True, stop=True,
            )
        nc.vector.tensor_copy(u2[g][:, 512 * t : 512 * (t + 1)], u_ps[:])

    # first q/k halves split into quarters so head 0 can start ~'10us in
    emit_load_q(ldq, qin, 0, 0, "q")
    emit_load_q(ldk, kin, 0, 0, "k")
    emit_load_q(ldq, qin, 0, 1, "q")
    emit_load_q(ldk, kin, 0, 1, "k")
    emit_qk_quarter(ldq[0], stq, qT, 0, 0, [nc.vector.tensor_copy, nc.gpsimd.tensor_copy])
    emit_qk_quarter(ldk[0], stk, kT, 0, 0, [nc.gpsimd.tensor_copy, nc.vector.tensor_copy])
    emit_u_t(0, 0)
    emit_qk_quarter(ldq[0], stq, qT, 0, 1, [nc.vector.tensor_copy, nc.gpsimd.tensor_copy])
    emit_qk_quarter(ldk[0], stk, kT, 0, 1, [nc.gpsimd.tensor_copy, nc.vector.tensor_copy])
    emit_u_t(0, 1)
    emit_u_t(1, 0)
    emit_u_t(1, 1)
    emit_load(ldv, vin, 0, "v")
    emit_load(ldq, qin, 1, "q")
    emit_load(ldk, kin, 1, "k")
    emit_v_half(0, ldv[0])
    emit_qk_half(ldq[1], stq, qT, 1, [nc.vector.tensor_copy, nc.gpsimd.tensor_copy])
    emit_qk_half(ldk[1], stk, kT, 1, [nc.gpsimd.tensor_copy, nc.vector.tensor_copy])
    emit_load(ldv, vin, 1, "v")
    emit_v_half(1, ldv[1])

    # ---------------- wo: cast-DMA + xbar transpose + Wv fold --------------
    wo_bf = persist.tile([128, 2, E], BF16, tag="wo_bf")
    nc.gpsimd.dma_start(
        out=wo_bf[:], in_=wo_s.rearrange("(rc p) e -> p rc e", p=128)
    )
    woT_t = persist.tile([128, 16, 128], BF16, tag="woT_t")
    nc.sync.dma_start_transpose(woT_t[:], wo_bf[:])
    # woT[c8][x, 128*rc + t] = woT_t[x, rc*8 + c8, t]
    woVT = persist.tile([128, 8, EOUT], BF16, tag="woVT")

    def emit_fold(c8):
        wv_ps = psB.tile([128, 2, 128], F32, tag="sc", name=f"wvps_{it}_{c8}")
        for hh in range(2):
            sl = slice(64 * hh, 64 * (hh + 1))
            nc.tensor.matmul(
                wv_ps[sl, :, :],
                wv2[sl, :],
                woT_t[sl, :, :].rearrange("p (rc c) t -> p c rc t", rc=2)[:, c8, :, :],
                start=True, stop=True,
            )
        nc.vector.tensor_copy(woVT[:, c8, :], wv_ps[:])

    # ---------------- AG staging ----------------
    in_cc = dram.tile([2 * Dh, S], BF16)  # heads 0,1
    in_cc2h = {
        (j, qh): dram.tile([Dh, S // 2], BF16, name=f"incc2_{it}_{j}_{qh}",
                           tag=f"incc2{j}{qh}")
        for j in (2, 3) for qh in range(2)
    }
    ag_outs = [
        dram.tile([512, S], BF16, addr_space="Local",
                  name=f"agout_{it}_{w}", tag=f"agout{w}")
        for w in range(2)
    ]
    ag2h = {
        (j, qh): dram.tile([4 * Dh, S // 2], BF16, addr_space="Local",
                           name=f"ag2h_{it}_{j}_{qh}", tag=f"ag2h{j}{qh}")
        for j in (2, 3) for qh in range(2)
    }
    agch = pool("agch", 1)
    cch = {
        c8: agch.tile([128, S], BF16, tag=f"ag{c8}", name=f"ag{c8}_{it}")
        for c8 in range(0, 8, 2)
    }
    cch_odd = [
        [agch.tile([128, S // 2], BF16, tag=f"agodd{r}_{h}", name=f"agodd{r}_{h}_{it}")
         for h in range(2)]
        for r in range(4)
    ]
    o_acc = [opool.tile([128, S], F32, tag=f"oacc{h}", bufs=1, name=f"oacc{h}_{it}")
             for h in range(2)]

    def emit_ag(which):
        if collective:
            nc.gpsimd.collective_compute(
                "AllGather",
                mybir.AluOpType.bypass,
                replica_groups=[[0, 1, 2, 3], [4, 5, 6, 7]],
                ins=[in_cc[:, :].opt()],
                outs=[ag_outs[which].opt()],
            )
        else:
            nc.sync.dma_start(out=ag_outs[which][0:128, :], in_=in_cc[:, :])
        for r in range(4):
            c8 = 2 * r + which
            nc.sync.dma_start(out=cch[c8][:], in_=ag_outs[which][128 * r : 128 * (r + 1), :])

    def emit_ag2(j, qh):
        """AllGather one head's q-half right after its normalize."""
        key = (j, qh)
        if collective:
            nc.gpsimd.collective_compute(
                "AllGather",
                mybir.AluOpType.bypass,
                replica_groups=[[0, 1, 2, 3], [4, 5, 6, 7]],
                ins=[in_cc2h[key][:, :].opt()],
                outs=[ag2h[key].opt()],
            )
        else:
            nc.sync.dma_start(out=ag2h[key][0:Dh, :], in_=in_cc2h[key][:, :])
        for r in range(4):
            nc.sync.dma_start(
                out=cch_odd[r][qh][Dh * (j - 2) : Dh * (j - 1), :],
                in_=ag2h[key][Dh * r : Dh * (r + 1), :],
            )

    # ---------------- attention ----------------
    w2_tiles = {}

    def emit_head_qh(j, qh, hooks):
        """scores -> exp -> (lagged) w2 accumulation for one (head, q-half)."""
        g, par = j // 2, j % 2
        sl = slice(64 * par, 64 * (par + 1))
        w2_ps = psW.tile([Dh + 1, S // 2], F32, tag="w2", name=f"w2ps_{it}_{j}_{qh}")
        w2_tiles[(j, qh)] = w2_ps
        p_tiles = {}

        def emit_w2(m):
            p_bf = p_tiles.pop(m)
            for u in range(2):
                nc.tensor.matmul(
                    w2_ps[:, 512 * u : 512 * (u + 1)],
                    vin_ones[:, m, j, :],
                    p_bf[:, 512 * u : 512 * (u + 1)],
                    start=(m == 0), stop=(m == NK - 1),
                )

        for m in range(NK):
            for f in hooks.get(m, ()):
                f()
            sc = psB.tile([128, S // 2], F32, tag="sc", name=f"sc_{it}_{j}_{qh}_{m}")
            for u in range(2):
                nc.tensor.matmul(
                    sc[:, 512 * u : 512 * (u + 1)],
                    kT[g][sl, m, :],
                    u2[g][sl, 1024 * qh + 512 * u : 1024 * qh + 512 * (u + 1)],
                    start=True, stop=True,
                )
            p_bf = ppool.tile([128, S // 2], BF16, tag="p", name=f"p_{it}_{j}_{qh}_{m}")
            p_tiles[m] = p_bf
            use_dve = ((m + 1) * DVE_EXP_N) // NK != (m * DVE_EXP_N) // NK
            if use_dve:
                nc.vector._custom_dve(
                    expop, out=p_bf[:], in0=sc[:], s0=EXP_C0, s1=EXP_C1,
                )
            else:
                nc.scalar.activation(
                    p_bf[:], sc[:], mybir.ActivationFunctionType.Exp,
                    scale=0.125, bias=nb1[:],
                )
            if m >= W2_LAG:
                emit_w2(m - W2_LAG)
        for m in range(NK - W2_LAG, NK):
            emit_w2(m)

    def emit_norm(j, qh):
        """reciprocal of the row-sum, broadcast, normalize, ship to DRAM."""
        w2_ps = w2_tiles.pop((j, qh))
        rs_sb = npool.tile([1, S // 2], F32, tag="rs_sb", name=f"rss_{it}_{j}_{qh}")
        nc.vector.tensor_copy(rs_sb[:], w2_ps[Dh : Dh + 1, :])
        rsr = npool.tile([1, S // 2], F32, tag="rsr", name=f"rsr_{it}_{j}_{qh}")
        nc.vector.reciprocal_approx_fast(out=rsr[:], in_=rs_sb[:])
        rs_b = npool.tile([Dh, S // 2], F32, tag="rs_b", name=f"rsb_{it}_{j}_{qh}")
        nc.gpsimd.partition_broadcast(rs_b[:], rsr[:])
        w2n = npool.tile([Dh, S // 2], BF16, tag="w2n", name=f"w2n_{it}_{j}_{qh}")
        nc.vector.tensor_mul(w2n[:], w2_ps[0:Dh, :], rs_b[:])
        if j < 2:
            nc.sync.dma_start(
                out=in_cc[Dh * j : Dh * (j + 1), 1024 * qh : 1024 * (qh + 1)],
                in_=w2n[:],
            )
        else:
            nc.sync.dma_start(
                out=in_cc2h[(j, qh)][:, :], in_=w2n[:]
            )

    def emit_oproj_group(round_, sh, h):
        o_ps = psB.tile([128, 1024], F32, tag="sc", name=f"ops_{it}_{round_}_{h}_{sh}")
        for i, r in enumerate(range(4)):
            c8 = 2 * r + round_
            for u in range(2):
                rhs = (
                    cch[c8][:, 1024 * sh + 512 * u : 1024 * sh + 512 * (u + 1)]
                    if round_ == 0
                    else cch_odd[r][sh][:, 512 * u : 512 * (u + 1)]
                )
                nc.tensor.matmul(
                    o_ps[:, 512 * u : 512 * (u + 1)],
                    woVT[:, c8, 128 * h : 128 * (h + 1)],
                    rhs,
                    start=(i == 0), stop=(i == 3),
                )
        if round_ == 0:
            nc.vector.tensor_copy(o_acc[h][:, 1024 * sh : 1024 * (sh + 1)], o_ps[:])
        else:
            o_sb = opool.tile([128, 1024], F32, tag="osb", name=f"osb_{it}_{h}_{sh}")
            nc.vector.scalar_tensor_tensor(
                o_sb[:], o_ps[:], bo_sb[:, h : h + 1],
                o_acc[h][:, 1024 * sh : 1024 * (sh + 1)],
                mybir.AluOpType.add, mybir.AluOpType.add,
            )
            nc.sync.dma_start(
                out=outT[128 * h : 128 * (h + 1), 1024 * sh : 1024 * (sh + 1)],
                in_=o_sb[:],
            )

    def emit_oproj(round_):
        for sh in range(2):
            for h in range(2):
                emit_oproj_group(round_, sh, h)

    # software pipeline: normalize of (j, qh) is emitted mid-way through the
    # following (head, q-half); AG0 follows head 1; oproj round 0 overlaps
    # head 3's first q-half.
    seq = [(j, qh) for j in range(HPC) for qh in range(2)]
    for idx, (j, qh) in enumerate(seq):
        hooks = {}
        if idx > 0:
            prev = seq[idx - 1]
            def mk(prev=prev):
                def f():
                    emit_norm(*prev)
                    if prev == (1, 1):
                        emit_ag(0)
                    if prev[0] >= 2:
                        emit_ag2(*prev)
                return f
            hooks[6] = [mk()]
        if (j, qh) == (0, 0):
            hooks[8] = [lambda: emit_u_t(0, 2)]
            hooks[10] = [lambda: emit_u_t(1, 2)]
            hooks[12] = [lambda: emit_u_t(0, 3)]
            hooks[14] = [lambda: emit_u_t(1, 3)]
        if (j, qh) == (0, 1):
            for i, m in enumerate((2, 5, 8, 11)):
                hooks.setdefault(m, []).append(
                    lambda i=i: (emit_fold(2 * i), emit_fold(2 * i + 1))
                )
        if (j, qh) == (3, 0):
            for i, m in enumerate((8, 10, 12, 14)):
                hooks.setdefault(m, []).append(
                    lambda i=i: emit_oproj_group(0, i // 2, i % 2)
                )
        emit_head_qh(j, qh, hooks)
    emit_norm(3, 1)
    emit_ag2(3, 1)
    for h in range(2):
        emit_oproj_group(1, 0, h)
    for h in range(2):
        emit_oproj_group(1, 1, h)


def _build(repeats=1, collective=True):
    key = (repeats, collective)
    if key in _CACHE:
        return _CACHE[key]
    _register_exp_op()
    ndev = N_CORES if collective else 1
    nc = bacc.Bacc("TRN2", target_bir_lowering=False, debug=False, num_devices=ndev)
    io = _declare_io(nc)
    with tile.TileContext(nc) as tc:
        for it in range(repeats):
            with contextlib.ExitStack() as es:
                _body(nc, tc, es, io, it, collective=collective)
    nc.compile()
    _CACHE[key] = nc
    return nc


def kernel(k_in, q_in, v_in, Wq, Wk, Wv, Wo, bo, _repeats=1, _results_hook=None):
    k_in = np.asarray(k_in, dtype=np.float32)
    q_in = np.asarray(q_in, dtype=np.float32)
    v_in = np.asarray(v_in, dtype=np.float32)
    Wq = np.ascontiguousarray(np.asarray(Wq, dtype=np.float32))
    Wk = np.ascontiguousarray(np.asarray(Wk, dtype=np.float32))
    Wv = np.ascontiguousarray(np.asarray(Wv, dtype=np.float32))
    Wo = np.asarray(Wo, dtype=np.float32)
    bo = np.asarray(bo, dtype=np.float32)

    nc = _build(_repeats)

    in_maps = []
    for c in range(N_CORES):
        b, q4 = c // 4, c % 4
        sl = slice(256 * q4, 256 * (q4 + 1))
        in_maps.append(
            {
                "qin": np.ascontiguousarray(q_in[b, :, sl]),
                "kin": np.ascontiguousarray(k_in[b, :, sl]),
                "vin": np.ascontiguousarray(v_in[b, :, sl]),
                "wq": Wq,
                "wk": Wk,
                "wv": Wv,
                "wo_s": np.ascontiguousarray(Wo[sl, :]),
                "bo_s": np.ascontiguousarray(bo[sl].reshape(2, 128)),
            }
        )

    res = run_bass_kernel_spmd(nc, in_maps, core_ids=list(range(N_CORES)))
    if _results_hook is not None:
        _results_hook(res)

    out = np.empty((B, S, E), dtype=np.float32)
    for c in range(N_CORES):
        b, q4 = c // 4, c % 4
        out[b, :, 256 * q4 : 256 * (q4 + 1)] = res.results[c]["outT"].T
    return out
